# revision 3
# baseline (speedup 1.0000x reference)
"""Trainium2 Bass kernel v2 for nn_LmLSTM: embedding -> 2x masked LSTM -> vocab projection.

Sharding: gate-sharded LSTM (core r owns hidden slice r of both layers),
one combined AllGather of both layers' h-shards per step (bf16). The
[H,V] projection is vocab-sharded and fully interleaved into the
AllGather wait gaps (1 n-tile per step), reading a windowed t-major h1
history assembled with one strided copy per step.

vs v1: input projections hoisted out of the recurrence (zx0 precomputed
as one big GEMM, bias folded; L1 bias injected via identity matmul),
gates packed [i,f,o,g] so the three sigmoids fuse into one activation
call, per-step staging DMAs moved to the sync engine, bf16 logits.
"""

import os
import sys
import types

import numpy as np
import ml_dtypes


def _install_axon_profile_hook():
    if "antenv.axon_hooks" in sys.modules:
        return
    holder = [None]
    mod = types.ModuleType("antenv.axon_hooks")
    mod.set_axon_ntff_profile_hook = lambda h: holder.__setitem__(0, h)
    mod.get_axon_ntff_profile_hook = lambda: holder[0]
    sys.modules["antenv.axon_hooks"] = mod
    try:
        import antenv

        antenv.axon_hooks = mod
        from trn_agent_boot.trn_boot import _ntff_profile_via_ctypes

        mod.set_axon_ntff_profile_hook(
            _ntff_profile_via_ctypes("/opt/axon/libaxon_pjrt.so")
        )
    except Exception:
        pass


_install_axon_profile_hook()

import concourse.bass as bass  # noqa: E402
import concourse.mybir as mybir  # noqa: E402
import concourse.tile as tile  # noqa: E402
from concourse.bass_utils import run_bass_kernel_spmd  # noqa: E402


def _install_wait_split():
    if getattr(bass.Bass, "_waitsplit_installed", False):
        return
    counter = [0]

    def _split(m):
        for f in m.functions:
            for bb in f.blocks:
                il = bb.instructions
                if not any(
                    i.sync_info is not None and len(i.sync_info.on_wait) > 1
                    for i in il
                ):
                    continue
                new = []
                for inst in il:
                    si = inst.sync_info
                    if si is not None and len(si.on_wait) > 1:
                        waits = list(si.on_wait)
                        si.on_wait = waits[:1]
                        for w in waits[1:]:
                            counter[0] += 1
                            nop = mybir.InstNoOp(
                                name=f"waitsplit_{counter[0]}", ins=[], outs=[]
                            )
                            nop.engine = inst.engine
                            nop.sync_info = mybir.SyncInfo(
                                on_wait=[w], on_update=[]
                            )
                            new.append(nop)
                    new.append(inst)
                il.clear()
                il.extend(new)

    orig = bass.Bass.to_json_bytes

    def patched(self, *a, **kw):
        _split(self.m)
        return orig(self, *a, **kw)

    bass.Bass.to_json_bytes = patched
    bass.Bass._waitsplit_installed = True


_install_wait_split()

# ---------------------------------------------------------------------------
V, E, H = 32000, 512, 1024
B = 16
T = int(os.environ.get("KERNEL_T", "256"))
NC = 8
VS = V // NC
NTOK = B * T
NTC = NTOK // 128  # 128-token chunks (t-major)
F32 = mybir.dt.float32
BF16 = mybir.dt.bfloat16
U8 = mybir.dt.uint8
SIG = mybir.ActivationFunctionType.Sigmoid
TANH = mybir.ActivationFunctionType.Tanh
COPY = mybir.ActivationFunctionType.Copy
GORD = [0, 1, 3, 2]  # packed gate order: i, f, o, g


def build_nc():
    nc = bass.Bass(num_devices=NC)
    d_w0 = nc.dram_tensor("w0t", [128, 8 * 4 * 128], BF16, kind="ExternalInput")
    d_w1 = nc.dram_tensor("w1t", [128, 16 * 4 * 128], BF16, kind="ExternalInput")
    d_wx0 = nc.dram_tensor("wx0", [128, 4 * 4 * 128], BF16, kind="ExternalInput")
    d_wout = nc.dram_tensor("woutp", [128, 8 * VS], BF16, kind="ExternalInput")
    d_b0t = nc.dram_tensor("b0t", [128, 4], F32, kind="ExternalInput")
    d_b1inj = nc.dram_tensor("b1inj", [128, 64], BF16, kind="ExternalInput")
    d_ident = nc.dram_tensor("ident", [128, 128], BF16, kind="ExternalInput")
    d_xt = nc.dram_tensor("xt", [E, NTOK], BF16, kind="ExternalInput")
    d_maskb = nc.dram_tensor("maskb", [128, NTOK], U8, kind="ExternalInput")
    d_mpt = nc.dram_tensor("maskpt", [128, NTC], F32, kind="ExternalInput")
    d_impt = nc.dram_tensor("invmpt", [128, NTC], F32, kind="ExternalInput")
    d_out = nc.dram_tensor("out", [NTOK, VS], BF16, kind="ExternalOutput")
    DBG = os.environ.get("KERNEL_DEBUG", "0") == "1"
    if DBG:
        d_dzx0 = nc.dram_tensor("dzx0", [128, 64 * T], BF16, kind="ExternalOutput")
        d_dh0 = nc.dram_tensor("dh0", [128, 128 * 4], BF16, kind="ExternalOutput")
        d_dh1 = nc.dram_tensor("dh1", [128, 128 * 4], BF16, kind="ExternalOutput")
        d_dhw = nc.dram_tensor("dhw", [128, 2048], BF16, kind="ExternalOutput")

    rg = [list(range(NC))]
    NPT = VS // 8  # 500 vocab cols per projection n-tile

    with tile.TileContext(nc) as tc:
        with (
            tc.tile_pool(name="wp", bufs=1) as wp,
            tc.tile_pool(name="sp", bufs=3) as sp,
            tc.tile_pool(name="pp", bufs=2, space="PSUM") as pp,
            tc.tile_pool(name="dp", bufs=3, space="DRAM") as dp,
        ):
            # ---- persistent loads ----
            w0t = wp.tile([128, 8 * 4 * 128], BF16, tag="w0t")
            w1t = wp.tile([128, 16 * 4 * 128], BF16, tag="w1t")
            wx0 = wp.tile([128, 4 * 4 * 128], BF16, tag="wx0")
            woutp = wp.tile([128, 8 * VS], BF16, tag="woutp")
            b0t = wp.tile([128, 4], F32, tag="b0t")
            b1inj = wp.tile([128, 64], BF16, tag="b1inj")
            ident = wp.tile([128, 128], BF16, tag="ident")
            maskb = wp.tile([128, NTOK], U8, tag="maskb")
            mpt = wp.tile([128, NTC], F32, tag="mpt")
            impt = wp.tile([128, NTC], F32, tag="impt")
            for dst, src in (
                (w0t, d_w0), (w1t, d_w1), (wx0, d_wx0), (b0t, d_b0t),
                (b1inj, d_b1inj), (ident, d_ident), (maskb, d_maskb),
                (mpt, d_mpt), (impt, d_impt),
            ):
                nc.gpsimd.dma_start(dst[:], src[:])
            nc.gpsimd.dma_start(woutp[:], d_wout[:])
            xt = []
            for k in range(4):
                xk = wp.tile([128, NTOK], BF16, tag=f"xt{k}")
                nc.gpsimd.dma_start(xk[:], d_xt[128 * k : 128 * (k + 1), :])
                xt.append(xk)

            # ---- persistent state ----
            # cg01: [tg1 | c1 | tg0 | c0]; hbf2: [h1 | h0] persistent bf16
            cg01 = wp.tile([128, 64], F32, tag="cg01")
            hbf2 = wp.tile([128, 32], BF16, tag="hbf2")
            for t_ in (cg01, hbf2):
                nc.vector.memset(t_[:], 0.0)
            cg0 = cg01[:, 32:64]
            cg1 = cg01[:, 0:32]
            # zx0' : hoisted Wx0@x + b0, packed [t][gate][batch] bf16
            zx0 = wp.tile([128, 64 * T], BF16, tag="zx0")
            # h1 history window (2 x 128 tokens, t-major, chunk-major)
            histw = wp.tile([128, 2048], BF16, tag="histw")
            zfull = wp.tile([128, 128], BF16, tag="zfull")
            nc.vector.memset(zfull[:], 0.0)
            h0full, h1full = zfull, zfull

            # ---- pre-phase: zx0' = Wx0.T @ x (+ b0) ----
            zx0v = zx0.rearrange("p (t g b) -> p t g b", g=4, b=16)
            for blk in range(T * B // 512):
                for g in range(4):
                    ps = pp.tile([128, 512], F32, tag="prep")
                    for k in range(4):
                        nc.tensor.matmul(
                            ps[:],
                            wx0[:, (k * 4 + g) * 128 : (k * 4 + g) * 128 + 128],
                            xt[k][:, 512 * blk : 512 * (blk + 1)],
                            start=(k == 0),
                            stop=(k == 3),
                        )
                    # strided store: cols (32*blk + t)*64 + g*16 + b
                    dst = zx0v[:, 32 * blk : 32 * (blk + 1), g, :]
                    nc.vector.tensor_scalar_add(
                        dst, ps.rearrange("p (t b) -> p t b", b=16), b0t[:, g : g + 1]
                    )

            def gate_chain(zps, cg, hbf_dst, mslice, tag):
                gsb = sp.tile([128, 48], F32, tag=f"gsb{tag}")
                nc.scalar.activation(gsb[:], zps[:, 0:48], SIG)
                nc.scalar.activation(cg[:, 0:16], zps[:, 48:64], TANH)
                tmp = sp.tile([128, 32], F32, tag=f"tmp{tag}")
                nc.vector.tensor_mul(tmp[:], gsb[:, 0:32], cg[:])  # i*g | f*c
                cn = sp.tile([128, 32], F32, tag=f"cn{tag}")
                nc.vector.tensor_add(cn[:, 0:16], tmp[:, 0:16], tmp[:, 16:32])
                nc.scalar.activation(cn[:, 16:32], cn[:, 0:16], TANH)
                hn = sp.tile([128, 16], BF16, tag=f"hn{tag}")
                nc.vector.tensor_mul(hn[:], gsb[:, 32:48], cn[:, 16:32])
                nc.vector.copy_predicated(cg[:, 16:32], mslice, cn[:, 0:16])
                nc.vector.copy_predicated(hbf_dst, mslice, hn[:])

            histv = histw.rearrange("p (w k s b) -> p w k s b", w=2, k=8, s=8)
            TEND = T + 1
            NITER = max(TEND, 8 * (NTC - 1) + 10 + 8)
            for t in range(NITER):
                s = t - 1  # layer-1 step
                if t < T:
                    # ---- layer 0, step t ----
                    z0 = pp.tile([128, 64], F32, tag="z0")
                    for g in range(4):
                        nc.tensor.matmul(
                            z0[:, 16 * g : 16 * g + 16],
                            ident[:],
                            zx0[:, 64 * t + 16 * g : 64 * t + 16 * g + 16],
                            start=True,
                            stop=False,
                        )
                        for k in range(8):
                            nc.tensor.matmul(
                                z0[:, 16 * g : 16 * g + 16],
                                w0t[:, (k * 4 + g) * 128 : (k * 4 + g) * 128 + 128],
                                h0full[:, 16 * k : 16 * k + 16],
                                start=False,
                                stop=(k == 7),
                            )
                    if not (1 <= t <= T - 1):
                        gate_chain(
                            z0, cg0, hbf2[:, 16:32],
                            maskb[:, 16 * t : 16 * t + 16], "0",
                        )
                if 1 <= t <= T:
                    # ---- layer 1, step s ----
                    z1 = pp.tile([128, 64], F32, tag="z1")
                    for g in range(4):
                        nc.tensor.matmul(
                            z1[:, 16 * g : 16 * g + 16],
                            ident[:],
                            b1inj[:, 16 * g : 16 * g + 16],
                            start=True,
                            stop=False,
                        )
                        for k in range(16):
                            rhs = (
                                h0full[:, 16 * k : 16 * k + 16]
                                if k < 8
                                else h1full[:, 16 * (k - 8) : 16 * (k - 8) + 16]
                            )
                            nc.tensor.matmul(
                                z1[:, 16 * g : 16 * g + 16],
                                w1t[:, (k * 4 + g) * 128 : (k * 4 + g) * 128 + 128],
                                rhs,
                                start=False,
                                stop=(k == 15),
                            )
                    if 1 <= t <= T - 1:
                        # fused two-layer chain, slots [L1 | L0]
                        gsb = sp.tile([128, 96], F32, tag="gsbf")
                        nc.scalar.activation(gsb[:, 0:48], z1[:, 0:48], SIG)
                        nc.scalar.activation(gsb[:, 48:96], z0[:, 0:48], SIG)
                        nc.scalar.activation(cg01[:, 0:16], z1[:, 48:64], TANH)
                        nc.scalar.activation(cg01[:, 32:48], z0[:, 48:64], TANH)
                        gv = gsb.rearrange("p (l c) -> p l c", l=2)
                        tmp = sp.tile([128, 64], F32, tag="tmpf")
                        nc.vector.tensor_mul(
                            tmp.rearrange("p (l c) -> p l c", l=2),
                            gv[:, :, 0:32],
                            cg01.rearrange("p (l c) -> p l c", l=2),
                        )
                        tv = tmp.rearrange("p (l c) -> p l c", l=2)
                        cn = sp.tile([128, 64], F32, tag="cnf")
                        nc.vector.tensor_add(
                            cn.rearrange("p (l c) -> p l c", c=16)[:, 0:2, :],
                            tv[:, :, 0:16], tv[:, :, 16:32],
                        )
                        nc.scalar.activation(cn[:, 32:64], cn[:, 0:32], TANH)
                        hn = sp.tile([128, 32], BF16, tag="hnf")
                        nc.vector.tensor_mul(
                            hn.rearrange("p (l c) -> p l c", l=2),
                            gv[:, :, 32:48],
                            cn.rearrange("p (l c) -> p l c", c=16)[:, 2:4, :],
                        )
                        m01 = maskb[:, 16 * s : 16 * s + 32]
                        nc.vector.copy_predicated(
                            cg01.rearrange("p (l c) -> p l c", c=16)[:, 1:4:2, :],
                            m01.rearrange("p (l c) -> p l c", l=2),
                            cn.rearrange("p (l c) -> p l c", c=16)[:, 0:2, :],
                        )
                        nc.vector.copy_predicated(hbf2[:], m01, hn[:])
                    else:
                        gate_chain(
                            z1, cg1, hbf2[:, 0:16],
                            maskb[:, 16 * s : 16 * s + 16], "1",
                        )

                # ---- h1 history: h1full currently holds h1(t-2) ----
                u = t - 2
                if 0 <= u < T:
                    dst = histv[:, (u // 8) % 2, :, u % 8, :]
                    nc.vector.tensor_copy(
                        dst, h1full.rearrange("p (k b) -> p k b", k=8)
                    )

                if t <= T:
                    # ---- combined AllGather of both shards ----
                    cc_in = dp.tile([256, 16], BF16, tag="cc_in")
                    cc_out = dp.tile([2048, 16], BF16, tag="cc_out")
                    nc.sync.dma_start(
                        cc_in.rearrange("(a p) b -> p a b", a=2),
                        hbf2.rearrange("p (a b) -> p a b", a=2),
                    )
                    nc.gpsimd.collective_compute(
                        "AllGather",
                        mybir.AluOpType.bypass,
                        ins=[cc_in.opt()],
                        outs=[cc_out.opt()],
                        replica_groups=rg,
                    )
                    h0new = sp.tile([128, 128], BF16, tag="h0full")
                    h1new = sp.tile([128, 128], BF16, tag="h1full")
                    v4 = cc_out.rearrange("(k a p) b -> a p k b", a=2, p=128)
                    nc.sync.dma_start(
                        h0new.rearrange("p (k b) -> p k b", k=8), v4[1]
                    )
                    nc.sync.dma_start(
                        h1new.rearrange("p (k b) -> p k b", k=8), v4[0]
                    )
                    h0full, h1full = h0new, h1new
                    if DBG and 1 <= t <= 4:
                        nc.gpsimd.dma_start(
                            d_dh0[:, 128 * (t - 1) : 128 * t], h0new[:]
                        )
                        nc.gpsimd.dma_start(
                            d_dh1[:, 128 * (t - 1) : 128 * t], h1new[:]
                        )
                    if DBG and t == 4:
                        nc.gpsimd.dma_start(d_dzx0[:], zx0[:])
                    if DBG and t == T:
                        nc.gpsimd.dma_start(d_dhw[:], histw[:])

                # ---- interleaved projection: chunk c, n-tile n ----
                pj = t - 10
                if pj >= 0 and (pj // 8) < NTC:
                    c, n = pj // 8, pj % 8
                    ps = pp.tile([128, NPT], F32, tag="proj")
                    for k in range(8):
                        nc.tensor.matmul(
                            ps[:],
                            histw[:, ((c % 2) * 1024) + 128 * k : ((c % 2) * 1024) + 128 * k + 128],
                            woutp[:, k * VS + n * NPT : k * VS + (n + 1) * NPT],
                            start=(k == 0),
                            stop=(k == 7),
                        )
                    if n == 0:
                        lgp = sp.tile([128, VS], BF16, tag="lgp", name="lgp")
                    nc.vector.tensor_scalar_mul(
                        lgp[:, n * NPT : (n + 1) * NPT], ps[:], mpt[:, c : c + 1]
                    )
                    if n == 0:
                        nc.vector.tensor_add(
                            lgp[:, 0:1], lgp[:, 0:1], impt[:, c : c + 1]
                        )
                    if n == 7:
                        nc.sync.dma_start(
                            d_out[128 * c : 128 * (c + 1), :], lgp[:]
                        )
    return nc


_NC_CACHE = [None]


def kernel(tokens, emb, Wx0, Wh0, b0, Wx1, Wh1, b1, Wout, bout):
    tokens = np.asarray(tokens)
    toks = tokens.astype(np.int64)
    emb = np.asarray(emb, np.float32)
    fm = (toks != 0).astype(np.float32)[:, :T]  # [B,T]

    x = emb[toks][:, :T]  # [B,T,E]
    xt = np.ascontiguousarray(x.transpose(2, 1, 0).reshape(E, NTOK)).astype(
        ml_dtypes.bfloat16
    )

    fm_flat = np.ascontiguousarray(fm.T).reshape(-1)  # t-major
    maskb = np.broadcast_to(fm_flat[None, :], (128, NTOK)).astype(np.uint8)
    mpt = np.ascontiguousarray(fm_flat.reshape(NTC, 128).T).astype(np.float32)
    impt_base = np.ascontiguousarray(
        (1.0 - fm_flat).reshape(NTC, 128).T
    ).astype(np.float32)

    Wx0a, Wh0a = np.asarray(Wx0, np.float32), np.asarray(Wh0, np.float32)
    Wx1a, Wh1a = np.asarray(Wx1, np.float32), np.asarray(Wh1, np.float32)
    Wouta = np.asarray(Wout, np.float32)
    b0a, b1a = np.asarray(b0, np.float32), np.asarray(b1, np.float32)
    bouta = np.asarray(bout, np.float32)

    def pack(w, nk):
        # w: [nk*128, 512slice] -> [128, nk*4*128], tile (k,g) at (k*4+g)*128
        a = w.reshape(nk, 128, 4, 128)
        return (
            np.ascontiguousarray(a.transpose(1, 0, 2, 3))
            .reshape(128, nk * 4 * 128)
            .astype(ml_dtypes.bfloat16)
        )

    ident = np.eye(128, dtype=np.float32).astype(ml_dtypes.bfloat16)

    in_maps = []
    for r in range(NC):
        cols = np.concatenate(
            [g * H + np.arange(128 * r, 128 * (r + 1)) for g in GORD]
        )
        w0 = Wh0a[:, cols]  # [1024, 512]
        w1 = np.concatenate([Wx1a[:, cols], Wh1a[:, cols]], 0)  # [2048, 512]
        wx0p = pack(Wx0a[:, cols], 4)
        wo = Wouta[:, VS * r : VS * (r + 1)]  # [1024, VS]
        woutp = (
            np.ascontiguousarray(wo.reshape(8, 128, VS).transpose(1, 0, 2))
            .reshape(128, 8 * VS)
            .astype(ml_dtypes.bfloat16)
        )
        b0s = b0a[cols].reshape(4, 128).T  # [128, 4] i,f,o,g
        b1s = b1a[cols].reshape(4, 128)  # [4, 128]
        b1inj = np.ascontiguousarray(
            np.broadcast_to(b1s[:, :, None], (4, 128, 16))
            .transpose(1, 0, 2)
            .reshape(128, 64)
        ).astype(ml_dtypes.bfloat16)
        in_maps.append(
            {
                "w0t": pack(w0, 8),
                "w1t": pack(w1, 16),
                "wx0": wx0p,
                "woutp": woutp,
                "b0t": np.ascontiguousarray(b0s),
                "b1inj": b1inj,
                "ident": ident,
                "xt": xt,
                "maskb": maskb,
                "maskpt": mpt,
                "invmpt": impt_base if r == 0 else np.zeros_like(impt_base),
            }
        )

    if _NC_CACHE[0] is None:
        _NC_CACHE[0] = build_nc()
    nc = _NC_CACHE[0]

    trace = os.environ.get("KERNEL_TRACE", "0") == "1"
    res = run_bass_kernel_spmd(
        nc, in_maps, core_ids=list(range(NC)), trace=trace
    )
    if trace and res.exec_time_ns is not None:
        print(f"HW exec time: {res.exec_time_ns} ns")

    logits = np.concatenate(
        [np.asarray(res.results[r]["out"]).astype(np.float32) for r in range(NC)],
        axis=1,
    )  # [NTOK(t-major), V]
    out = logits.reshape(T, B, V).transpose(1, 0, 2)
    if np.any(bouta):
        out = out + bouta[None, None, :] * fm[:, :, None]
    out = np.ascontiguousarray(out, np.float32)
    if T < tokens.shape[1]:
        full = np.zeros((B, tokens.shape[1], V), np.float32)
        full[:, :T] = out
        out = full
    return out
